# revision 1
# baseline (speedup 1.0000x reference)
"""DISCO downsample conv (3x3, stride 2, pad 1) on 8 Trainium2 NeuronCores.

Strategy:
  - Effective weights w[o,i,kh,kw] = sum_b coeff[o,i,b]*basis[b,kh,kw] are tiny:
    computed on host, shipped per-tap transposed as wt[i, tap, o] (fp16),
    pre-scaled by 1/XSCALE to undo the input scaling for free.
  - x is zero-padded (H+2, W+2) on host, W phase-split into [even | odd]
    columns so every conv tap reads a contiguous run of 256 columns, scaled
    by XSCALE=4 (centers N(0,1) data in fp8-e3m4's narrow normal range) and
    cast to fp8 e3m4 (4-bit mantissa, rel err ~1.26e-2 on this conv). The
    matmul streams the e3m4 moving operand at the same 1 row/cycle as fp16
    while the stationary weights stay fp16, so TensorE time is unchanged but
    input HBM traffic halves -- the DMA engines drop well below the TensorE
    streaming floor (64 row-pairs x 9 taps x 512 px @ 2.4GHz = ~125us),
    which is the binding constraint for this shape.
    (fp8 DoubleRow tap-pairing was tried and reverted: a DoubleRow matmul
    following any normal-mode matmul wedges the PE -- hardware hang -- and
    with the 2e-2 accuracy gate nothing cheaper than 9 fp16-rate tap matmuls
    per row-pair survives.)
  - Sharding: 8 shards = (batch b in 0..3) x (H half in 0..1). Each core gets
    padded rows [256*h, 256*h + 257) of batch b -- the 1-row halo is part of
    the shard, so no inter-core communication is needed.
  - Startup: no DMA byte can land before ~9us (engine preamble ~6us + queue
    arm ~3us), so TensorE runs dummy matmuls on a memset tile in that dead
    window to pre-warm the HAM clock gate; without it the first ~6us of real
    matmuls run at half clock. The dummy chain is sized to end right as the
    first x chunk lands -- a gap between dummies and real work resets the
    ramp.
  - Per core: 4 row-blocks (32 output rows each = 65 input rows), block 0
    loaded in fine chunks so compute starts as soon as the first rows land,
    later blocks in 2 big chunks (per-dma_start ~2us re-arm amortized). For
    each pair of output rows: one PSUM bank [96, 512], 9 accumulating
    matmuls, then ScalarE adds bias while copying PSUM -> SBUF (fp16), and
    rows flush to HBM on the ACT HWDGE ring (kept warm all kernel; the idle
    SP ring pays a fresh queue-arm per flush). The last block flushes in
    quarters and its final two rows become single-row N=256 tiles, so the
    end-of-kernel chain after the last matmul is one small ACT + one 49KB
    flush + the fixed completion protocol.
"""

import os
import sys
import types

import numpy as np


# ----------------------------------------------------------------------------
# Environment bootstrap (self-contained: no reads from /root/problem).
# ----------------------------------------------------------------------------
def _ensure_paths():
    for p in (
        "/root/.axon_site",
        "/root/.axon_site/_ro/trn_rl_repo",
        "/root/.axon_site/_ro/pypackages",
        "/opt/trn_rl_repo",
    ):
        if os.path.isdir(p) and p not in sys.path:
            sys.path.append(p)


_ensure_paths()

import ml_dtypes  # noqa: E402


def _install_ntff_hook():
    """Register the NTFF profile hook (used when tracing; harmless otherwise)."""
    try:
        import antenv
    except ImportError:
        return
    if "antenv.axon_hooks" not in sys.modules:
        hooks_mod = types.ModuleType("antenv.axon_hooks")
        _hook = [None]
        hooks_mod.set_axon_ntff_profile_hook = lambda h: _hook.__setitem__(0, h)
        hooks_mod.get_axon_ntff_profile_hook = lambda: _hook[0]
        sys.modules["antenv.axon_hooks"] = hooks_mod
        antenv.axon_hooks = hooks_mod
    from antenv.axon_hooks import (
        get_axon_ntff_profile_hook,
        set_axon_ntff_profile_hook,
    )

    if get_axon_ntff_profile_hook() is None:
        try:
            from trn_agent_boot.trn_boot import _ntff_profile_via_ctypes

            so = "/opt/axon/libaxon_pjrt.so"
            if os.path.exists(so):
                set_axon_ntff_profile_hook(_ntff_profile_via_ctypes(so))
        except Exception:
            pass


_install_ntff_hook()

import concourse.bass as bass  # noqa: E402
import concourse.tile as tile  # noqa: E402
from concourse import bacc, mybir  # noqa: E402
import concourse.bass_utils as _bu  # noqa: E402

# Artifact upload needs a bucket that isn't reachable here; keep traces local.
_bu.upload_artifacts = lambda tmpdir: f"local:{tmpdir}"

XDT = mybir.dt.float8e3     # moving operand: fp8 e3m4 (4-bit mantissa)
WDT = mybir.dt.float16      # stationary weights stay fp16
F16 = mybir.dt.float16
F32 = mybir.dt.float32
NP_XDT = ml_dtypes.float8_e3m4
XSCALE = 4.0                # x *= 4 fits N(0,1) in e3m4 normals; w /= 4
XCLIP = 15.5                # e3m4 max finite

C = 96          # channels (in == out)
K = 3           # kernel size
N_CORES = 8
H = W = 512     # input spatial
HO = WO = 256   # output spatial
HP = H + 2      # padded rows
WP = W + 2      # padded cols (phase-split: [257 even | 257 odd])
SH_ROWS = 257   # padded rows per shard (256 + 1 halo)
CORE_HO = 128   # output rows per core
BH = 32         # output rows per block
NBLK = CORE_HO // BH
IN_ROWS = 2 * BH + 1  # input rows per block (65)
N_WARM = 26     # HAM pre-warm dummy matmuls

# column base per kw tap: even-phase col 2*ow -> slot ow (base 0);
# odd-phase col 2*ow+1 -> slot 257+ow; even col 2*ow+2 -> slot ow+1.
_KW_BASE = {0: 0, 1: 257, 2: 1}

_PROGRAM_CACHE = {}


def _build_program():
    """One SPMD Bass program, shared by all 8 cores."""
    nc = bacc.Bacc()
    x_d = nc.dram_tensor("x", [C, SH_ROWS, WP], XDT, kind="ExternalInput")
    w_d = nc.dram_tensor("wt", [C, K * K, C], WDT, kind="ExternalInput")
    b_d = nc.dram_tensor("bias", [C, 1], F32, kind="ExternalInput")
    y_d = nc.dram_tensor("out", [C, CORE_HO, WO], F16, kind="ExternalOutput")

    with tile.TileContext(nc) as tc:
        with (
            tc.tile_pool(name="const", bufs=1) as cpool,
            tc.tile_pool(name="xin", bufs=2) as xpool,
            tc.tile_pool(name="oout", bufs=2) as opool,
            tc.tile_pool(name="psum", bufs=8, space=bass.MemorySpace.PSUM) as ppool,
        ):
            # HAM pre-warm source: a zeroed fp16 tile, matmul'd into junk PSUM
            # while the first x chunk is still in DMA-queue-arm limbo.
            warm = cpool.tile([C, 352], F16)
            nc.vector.memset(warm[:], 0.0)
            # constants ride SWDGE: it emits right after the engine preamble
            # (~6us) and lands the small wt before the first x chunk arrives
            wt = cpool.tile([C, K * K, C], WDT)
            nc.gpsimd.dma_start(wt[:], w_d[:])
            bias = cpool.tile([C, 1], F32)
            nc.gpsimd.dma_start(bias[:], b_d[:])

            # warm-up matmuls use full-size PSUM tiles so they don't perturb
            # the pool's bank packing for the real accumulators
            for i in range(N_WARM):
                wps = ppool.tile([C, 2 * WO], F32, name="wps", tag="ps")
                nc.tensor.matmul(
                    wps[:, 0:256], warm[:, 0:C], warm[:, C : C + 256],
                    start=True, stop=True,
                )

            for blk in range(NBLK):
                xt = xpool.tile([C, IN_ROWS, WP], XDT, name="xt", tag="xt")
                chunks = (5, 13, 15, 16, 16) if blk == 0 else (33, 32)
                r0 = 2 * BH * blk
                rr = 0
                for nrows in chunks:
                    nc.sync.dma_start(
                        xt[:, rr : rr + nrows, :],
                        x_d[:, r0 + rr : r0 + rr + nrows, :],
                    )
                    rr += nrows
                assert rr == IN_ROWS
                out_sb = opool.tile([C, BH, WO], F16)
                last = blk == NBLK - 1
                # last block: final two rows become single-row N=256 tiles
                # below, halving the end-of-kernel ACT+flush chain
                for t in range(BH // 2 - 1 if last else BH // 2):
                    ps = ppool.tile([C, 2 * WO], F32, name="ps", tag="ps")
                    for tap in range(K * K):
                        kh, kw = tap // K, tap % K
                        cb = _KW_BASE[kw]
                        rhs = xt[:, 4 * t + kh : 4 * t + kh + 3 : 2, cb : cb + WO]
                        nc.tensor.matmul(
                            ps[:],
                            wt[:, tap, :],
                            rhs,
                            start=(tap == 0),
                            stop=(tap == K * K - 1),
                        )
                    nc.scalar.activation(
                        out_sb[:, 2 * t : 2 * t + 2, :],
                        ps[:].rearrange("p (a b) -> p a b", a=2),
                        mybir.ActivationFunctionType.Identity,
                        bias=bias[:],
                    )
                    # flush finished rows on the ACT HWDGE ring as soon as they
                    # complete; the last block flushes in quarters to shrink
                    # the kernel tail
                    flush_at = (
                        (7, 11, 13, 14) if last else (BH // 4 - 1, BH // 2 - 1)
                    )
                    if t in flush_at:
                        fi = flush_at.index(t)
                        prev = 0 if fi == 0 else (flush_at[fi - 1] + 1)
                        lo, hi = 2 * prev, 2 * t + 2
                        nc.scalar.dma_start(
                            y_d[:, BH * blk + lo : BH * blk + hi, :],
                            out_sb[:, lo:hi, :],
                        )
                if last:
                    for r in (BH - 2, BH - 1):
                        # full-bank PSUM tile (uniform 2KB allocations keep
                        # every accumulator bank-aligned; mixed sizes let the
                        # pool pack 2KB tiles across bank boundaries -> every
                        # matmul slows 217->260ns, and the packing varies
                        # from compile to compile)
                        psrf = ppool.tile([C, 2 * WO], F32, name="psrf", tag="ps")
                        psr = psrf[:, 0:WO]
                        for tap in range(K * K):
                            kh, kw = tap // K, tap % K
                            cb = _KW_BASE[kw]
                            nc.tensor.matmul(
                                psr[:],
                                wt[:, tap, :],
                                xt[:, 2 * r + kh, cb : cb + WO],
                                start=(tap == 0),
                                stop=(tap == K * K - 1),
                            )
                        nc.scalar.activation(
                            out_sb[:, r, :],
                            psr[:],
                            mybir.ActivationFunctionType.Identity,
                            bias=bias[:],
                        )
                        nc.scalar.dma_start(
                            y_d[:, BH * blk + r, :], out_sb[:, r, :]
                        )

    nc.compile()
    return nc


def _get_program():
    if "nc" not in _PROGRAM_CACHE:
        _PROGRAM_CACHE["nc"] = _build_program()
    return _PROGRAM_CACHE["nc"]


def _prepare_inputs(x, coeff, basis, bias):
    """Host prep: effective weights, padded phase-split e3m4 x, shards."""
    x = np.asarray(x)
    coeff = np.asarray(coeff)
    basis = np.asarray(basis)
    bias = np.asarray(bias)
    B = coeff.shape[2]
    # wt[i, tap, o] = sum_b coeff[o,i,b] * basis[b, tap], pre-scaled 1/XSCALE
    w_eff = (
        coeff.astype(np.float32).reshape(C * C, B)
        @ basis.astype(np.float32).reshape(B, K * K)
    ).reshape(C, C, K * K)
    wt = np.ascontiguousarray(
        w_eff.transpose(1, 2, 0) * np.float32(1.0 / XSCALE)
    ).astype(np.float16)

    xs = np.clip(x.astype(np.float32) * np.float32(XSCALE), -XCLIP, XCLIP)
    xb = xs.astype(NP_XDT)
    xph = np.zeros((x.shape[0], C, HP, WP), dtype=NP_XDT)
    # even phase: padded col 2j -> orig col 2j-1  (slot j=1..256)
    xph[:, :, 1 : H + 1, 1:257] = xb[:, :, :, 1::2]
    # odd phase: padded col 2j+1 -> orig col 2j  (slot 257+j, j=0..255)
    xph[:, :, 1 : H + 1, 257:513] = xb[:, :, :, 0::2]

    bias2 = np.ascontiguousarray(bias.astype(np.float32).reshape(C, 1))

    in_maps = []
    for s in range(N_CORES):
        b_idx, h_idx = divmod(s, 2)
        shard = np.ascontiguousarray(
            xph[b_idx, :, 256 * h_idx : 256 * h_idx + SH_ROWS, :]
        )
        in_maps.append({"x": shard, "wt": wt, "bias": bias2})
    return in_maps


def _assemble(results, n_batch):
    out = np.empty((n_batch, C, 2 * CORE_HO, WO), dtype=np.float32)
    for s in range(N_CORES):
        b_idx, h_idx = divmod(s, 2)
        out[b_idx, :, CORE_HO * h_idx : CORE_HO * (h_idx + 1), :] = results[s][
            "out"
        ].astype(np.float32)
    return out


def run(x, coeff, basis, bias, trace=False, trace_cores=None):
    """Run the kernel; returns (full_output, BassKernelResults)."""
    nc = _get_program()
    in_maps = _prepare_inputs(x, coeff, basis, bias)
    last_err = None
    for attempt in range(3):
        try:
            res = _bu.run_bass_kernel_spmd(
                nc,
                in_maps,
                list(range(N_CORES)),
                trace=trace,
                trace_cores=trace_cores,
            )
            return _assemble(res.results, x.shape[0]), res
        except Exception as e:  # transient NRT device-unrecoverable after
            last_err = e        # abrupt neighbor-process exits; nudge + retry
            if attempt == 2 or "UNAVAILABLE" not in str(e):
                raise
            import time

            import jax
            import jax.numpy as jnp

            time.sleep(15)
            try:
                a = jnp.ones((8, 8))
                (a @ a).block_until_ready()
            except Exception:
                time.sleep(15)
    raise last_err


def kernel(x, coeff, basis, bias):
    out, _ = run(x, coeff, basis, bias, trace=False)
    return out



# revision 2
# speedup vs baseline: 1.0565x; 1.0565x over previous
"""DISCO downsample conv (3x3, stride 2, pad 1) on 8 Trainium2 NeuronCores.

v4: K=128-packed matmuls — 8 matmuls per output row-pair instead of 9.

Baseline scheme: 9 tap-matmuls of K=96 (in-channels) per output row-pair;
TensorE streams 9 x 512 px = 4608 cycles/pair -> ~123us floor, PE only
96/128 partitions busy.

v4 packs one tap -- (2,2) -- into SBUF partitions 96-127 as host-prepared
shifted 32-channel copies ("helper"), so 3 of the matmuls contract
(96 ch x host tap) + (32 ch x a (2,2)-chunk) simultaneously: 3 x K=128 +
5 x K=96 = 8 matmuls/pair = 4096 cycles -> ~109us TensorE floor.

Why only one distributed tap: every matmul on the shared x tile reads at
byte offsets fixed by (kh row, kw column-base); with the pitch-514 phase
layout [evenPh 257 | oddPh 257], bases are {0, 257, 1} and kw=0/kw=2
ranges overlap, leaving exactly 4 non-colliding helper slots (2 even-row
+ 2 odd-row) -> 3 chunks of one tap. (A 7-matmul variant with 6 slots
needs a pitch-771 layout whose extra traffic / strided DMA destinations
measured slower: per-core DMA saturates at ~265 GB/s aggregate and
non-contiguous destinations collapse to tiny descriptors at <100 GB/s.)

Traffic: main x 12.7MB + helper 6.3MB + out 6.3MB per core, all
contiguous-destination transfers -> ~96us, under the 109us floor.

Numerics identical to baseline (same fp8 e3m4 x, fp16 w products, fp32
PSUM accumulation -- just regrouped), rel err ~1.26e-2.

Sharding: 8 shards = (batch 0..3) x (H half 0..1), 1-row halo included,
no inter-core communication. Startup warm-up matmuls, fine-grained
block-0 chunks, ACT-ring output flushes, and quartered last-block tail
kept from baseline.
"""

import os
import sys
import types

import numpy as np


# ----------------------------------------------------------------------------
# Environment bootstrap (self-contained: no reads from /root/problem).
# ----------------------------------------------------------------------------
def _ensure_paths():
    for p in (
        "/root/.axon_site",
        "/root/.axon_site/_ro/trn_rl_repo",
        "/root/.axon_site/_ro/pypackages",
        "/opt/trn_rl_repo",
    ):
        if os.path.isdir(p) and p not in sys.path:
            sys.path.append(p)


_ensure_paths()

import ml_dtypes  # noqa: E402


def _install_ntff_hook():
    """Register the NTFF profile hook (used when tracing; harmless otherwise)."""
    try:
        import antenv
    except ImportError:
        return
    if "antenv.axon_hooks" not in sys.modules:
        hooks_mod = types.ModuleType("antenv.axon_hooks")
        _hook = [None]
        hooks_mod.set_axon_ntff_profile_hook = lambda h: _hook.__setitem__(0, h)
        hooks_mod.get_axon_ntff_profile_hook = lambda: _hook[0]
        sys.modules["antenv.axon_hooks"] = hooks_mod
        antenv.axon_hooks = hooks_mod
    from antenv.axon_hooks import (
        get_axon_ntff_profile_hook,
        set_axon_ntff_profile_hook,
    )

    if get_axon_ntff_profile_hook() is None:
        try:
            from trn_agent_boot.trn_boot import _ntff_profile_via_ctypes

            so = "/opt/axon/libaxon_pjrt.so"
            if os.path.exists(so):
                set_axon_ntff_profile_hook(_ntff_profile_via_ctypes(so))
        except Exception:
            pass


_install_ntff_hook()

import concourse.bass as bass  # noqa: E402
import concourse.tile as tile  # noqa: E402
from concourse import bacc, mybir  # noqa: E402
import concourse.bass_utils as _bu  # noqa: E402

# Artifact upload needs a bucket that isn't reachable here; keep traces local.
_bu.upload_artifacts = lambda tmpdir: f"local:{tmpdir}"

XDT = mybir.dt.float8e3     # moving operand: fp8 e3m4 (4-bit mantissa)
WDT = mybir.dt.float16      # stationary weights stay fp16
F16 = mybir.dt.float16
F32 = mybir.dt.float32
NP_XDT = ml_dtypes.float8_e3m4
XSCALE = 4.0                # x *= 4 fits N(0,1) in e3m4 normals; w /= 4
XCLIP = 15.5                # e3m4 max finite

C = 96          # channels (in == out)
K = 3           # kernel size
N_CORES = 8
H = W = 512     # input spatial
HO = WO = 256   # output spatial
HP = H + 2      # padded rows
PITCH = 514     # cols: [evenPh 257 | oddPh 257]
SH_ROWS = 257   # padded rows per shard (256 + 1 halo)
CORE_HO = 128   # output rows per core
BH = 32         # output rows per block
NBLK = CORE_HO // BH
IN_ROWS = 2 * BH + 1  # input rows per block (65)
N_WARM = 26     # HAM pre-warm dummy matmuls

# 8-matmul roster per row-pair: (kh, col_base, K_partitions, spare).
# col_base: kw=0 -> 0 (evenPh), kw=1 -> 257 (oddPh), kw=2 -> 1 (evenPh).
# K=128 entries carry a 32-ch chunk of the distributed tap (2,2) on
# partitions 96-127; spare = (tap, ch0) weight rows for those.
MM_ROSTER = (
    (0, 0, 96, None),        # (0,0)
    (0, 257, 128, (8, 32)),  # (0,1) + tap(2,2) ch 32-63
    (0, 1, 128, (8, 0)),     # (0,2) + tap(2,2) ch 0-31
    (1, 0, 96, None),        # (1,0)
    (1, 257, 128, (8, 64)),  # (1,1) + tap(2,2) ch 64-95
    (1, 1, 96, None),        # (1,2)
    (2, 0, 96, None),        # (2,0)
    (2, 257, 96, None),      # (2,1)
)
HOST_TAPS = (0, 1, 2, 3, 4, 5, 6, 7)  # tap index kh*3+kw per roster entry

_PROGRAM_CACHE = {}


def _build_program():
    """One SPMD Bass program, shared by all 8 cores."""
    nc = bacc.Bacc()
    x_d = nc.dram_tensor("x", [C, SH_ROWS, PITCH], XDT, kind="ExternalInput")
    h_d = nc.dram_tensor("hx", [32, SH_ROWS, PITCH], XDT, kind="ExternalInput")
    w_d = nc.dram_tensor("wt", [128, 8, C], WDT, kind="ExternalInput")
    b_d = nc.dram_tensor("bias", [C, 1], F32, kind="ExternalInput")
    y_d = nc.dram_tensor("out", [C, CORE_HO, WO], F16, kind="ExternalOutput")

    with tile.TileContext(nc) as tc:
        with (
            tc.tile_pool(name="const", bufs=1) as cpool,
            tc.tile_pool(name="xin", bufs=2) as xpool,
            tc.tile_pool(name="oout", bufs=2) as opool,
            tc.tile_pool(name="psum", bufs=8, space=bass.MemorySpace.PSUM) as ppool,
        ):
            # HAM pre-warm source: a zeroed fp16 tile, matmul'd into junk PSUM
            # while the first x chunk is still in DMA-queue-arm limbo.
            warm = cpool.tile([C, 352], F16)
            nc.vector.memset(warm[:], 0.0)
            # constants ride SWDGE: it emits right after the engine preamble
            # (~6us) and lands the small wt before the first x chunk arrives
            wt = cpool.tile([128, 8, C], WDT)
            nc.gpsimd.dma_start(wt[:], w_d[:])
            bias = cpool.tile([C, 1], F32)
            nc.gpsimd.dma_start(bias[:], b_d[:])

            # warm-up matmuls use full-size PSUM tiles so they don't perturb
            # the pool's bank packing for the real accumulators
            for i in range(N_WARM):
                wps = ppool.tile([C, 2 * WO], F32, name="wps", tag="ps")
                nc.tensor.matmul(
                    wps[:, 0:256], warm[:, 0:C], warm[:, C : C + 256],
                    start=True, stop=True,
                )

            for blk in range(NBLK):
                xt = xpool.tile([128, IN_ROWS, PITCH], XDT, name="xt", tag="xt")
                r0 = 2 * BH * blk
                # main channels (partitions 0-95) on the sync HWDGE ring
                chunks = (5, 13, 15, 16, 16) if blk == 0 else (33, 32)
                rr = 0
                for nrows in chunks:
                    nc.sync.dma_start(
                        xt[0:C, rr : rr + nrows, :],
                        x_d[:, r0 + rr : r0 + rr + nrows, :],
                    )
                    rr += nrows
                assert rr == IN_ROWS
                # helper partitions 96-127 on the gpsimd SWDGE ring
                hchunks = (5, 28, 32) if blk == 0 else (33, 32)
                rr = 0
                for nrows in hchunks:
                    nc.gpsimd.dma_start(
                        xt[C:128, rr : rr + nrows, :],
                        h_d[:, r0 + rr : r0 + rr + nrows, :],
                    )
                    rr += nrows
                assert rr == IN_ROWS
                out_sb = opool.tile([C, BH, WO], F16)
                last = blk == NBLK - 1
                # last block: final two rows become single-row N=256 tiles
                # below, halving the end-of-kernel ACT+flush chain
                for t in range(BH // 2 - 1 if last else BH // 2):
                    ps = ppool.tile([C, 2 * WO], F32, name="ps", tag="ps")
                    for k, (kh, cb, kp, _sp) in enumerate(MM_ROSTER):
                        rhs = xt[0:kp, 4 * t + kh : 4 * t + kh + 3 : 2, cb : cb + WO]
                        nc.tensor.matmul(
                            ps[:],
                            wt[0:kp, k, :],
                            rhs,
                            start=(k == 0),
                            stop=(k == len(MM_ROSTER) - 1),
                        )
                    nc.scalar.activation(
                        out_sb[:, 2 * t : 2 * t + 2, :],
                        ps[:].rearrange("p (a b) -> p a b", a=2),
                        mybir.ActivationFunctionType.Identity,
                        bias=bias[:],
                    )
                    # flush finished rows on the ACT HWDGE ring as soon as they
                    # complete; the last block flushes in quarters to shrink
                    # the kernel tail
                    flush_at = (
                        (7, 11, 13, 14) if last else (BH // 4 - 1, BH // 2 - 1)
                    )
                    if t in flush_at:
                        fi = flush_at.index(t)
                        prev = 0 if fi == 0 else (flush_at[fi - 1] + 1)
                        lo, hi = 2 * prev, 2 * t + 2
                        nc.scalar.dma_start(
                            y_d[:, BH * blk + lo : BH * blk + hi, :],
                            out_sb[:, lo:hi, :],
                        )
                if last:
                    for r in (BH - 2, BH - 1):
                        # full-bank PSUM tile (uniform 2KB allocations keep
                        # every accumulator bank-aligned)
                        psrf = ppool.tile([C, 2 * WO], F32, name="psrf", tag="ps")
                        psr = psrf[:, 0:WO]
                        for k, (kh, cb, kp, _sp) in enumerate(MM_ROSTER):
                            nc.tensor.matmul(
                                psr[:],
                                wt[0:kp, k, :],
                                xt[0:kp, 2 * r + kh, cb : cb + WO],
                                start=(k == 0),
                                stop=(k == len(MM_ROSTER) - 1),
                            )
                        nc.scalar.activation(
                            out_sb[:, r, :],
                            psr[:],
                            mybir.ActivationFunctionType.Identity,
                            bias=bias[:],
                        )
                        nc.scalar.dma_start(
                            y_d[:, BH * blk + r, :], out_sb[:, r, :]
                        )

    nc.compile()
    return nc


def _get_program():
    if "nc" not in _PROGRAM_CACHE:
        _PROGRAM_CACHE["nc"] = _build_program()
    return _PROGRAM_CACHE["nc"]


def _prepare_inputs(x, coeff, basis, bias):
    """Host prep: effective weights, phase-split e3m4 x + helper, shards."""
    x = np.asarray(x)
    coeff = np.asarray(coeff)
    basis = np.asarray(basis)
    bias = np.asarray(bias)
    B = coeff.shape[2]
    NB = x.shape[0]
    # wq[i, tap, o] = sum_b coeff[o,i,b] * basis[b, tap], pre-scaled 1/XSCALE
    w_eff = (
        coeff.astype(np.float32).reshape(C * C, B)
        @ basis.astype(np.float32).reshape(B, K * K)
    ).reshape(C, C, K * K)
    wq = np.ascontiguousarray(
        w_eff.transpose(1, 2, 0) * np.float32(1.0 / XSCALE)
    ).astype(np.float16)
    wt128 = np.zeros((128, 8, C), np.float16)
    for k, (kh, cb, kp, spare) in enumerate(MM_ROSTER):
        wt128[0:C, k, :] = wq[:, HOST_TAPS[k], :]
        if spare is not None:
            stap, ch0 = spare
            wt128[C:128, k, :] = wq[ch0 : ch0 + 32, stap, :]

    xs = np.clip(x.astype(np.float32) * np.float32(XSCALE), -XCLIP, XCLIP)
    xq = xs.astype(NP_XDT)
    # phases over padded cols: A[j] = padded col 2j, Bp[j] = padded col 2j+1
    A = np.zeros((NB, C, HP, 257), dtype=NP_XDT)
    Bp = np.zeros((NB, C, HP, 257), dtype=NP_XDT)
    A[:, :, 1 : H + 1, 1:257] = xq[:, :, :, 1::2]   # padded col 2j = orig 2j-1
    Bp[:, :, 1 : H + 1, 0:256] = xq[:, :, :, 0::2]  # padded col 2j+1 = orig 2j

    xph = np.zeros((NB, C, HP, PITCH), dtype=NP_XDT)
    xph[..., 0:257] = A
    xph[..., 257:514] = Bp

    # helper tensor: 3 slots carrying tap (2,2) chunks (see module doc)
    hlp = np.zeros((NB, 32, HP, PITCH), dtype=NP_XDT)
    ev = np.arange(0, HP, 2)
    ev2 = ev[ev + 2 < HP]
    od = np.arange(1, HP, 2)
    od1 = od[od + 1 < HP]
    hlp[:, :, ev2, 1:257] = A[:, 0:32, ev2 + 2, 1:257]     # (0,2) <- tap8 ch0
    hlp[:, :, ev2, 257:513] = A[:, 32:64, ev2 + 2, 1:257]  # (0,1) <- tap8 ch32
    hlp[:, :, od1, 257:513] = A[:, 64:96, od1 + 1, 1:257]  # (1,1) <- tap8 ch64

    bias2 = np.ascontiguousarray(bias.astype(np.float32).reshape(C, 1))

    in_maps = []
    for s in range(N_CORES):
        b_idx, h_idx = divmod(s, 2)
        shard = np.ascontiguousarray(
            xph[b_idx, :, 256 * h_idx : 256 * h_idx + SH_ROWS, :]
        )
        hshard = np.ascontiguousarray(
            hlp[b_idx, :, 256 * h_idx : 256 * h_idx + SH_ROWS, :]
        )
        in_maps.append(
            {"x": shard, "hx": hshard, "wt": wt128, "bias": bias2}
        )
    return in_maps


def _assemble(results, n_batch):
    out = np.empty((n_batch, C, 2 * CORE_HO, WO), dtype=np.float32)
    for s in range(N_CORES):
        b_idx, h_idx = divmod(s, 2)
        out[b_idx, :, CORE_HO * h_idx : CORE_HO * (h_idx + 1), :] = results[s][
            "out"
        ].astype(np.float32)
    return out


def run(x, coeff, basis, bias, trace=False, trace_cores=None):
    """Run the kernel; returns (full_output, BassKernelResults)."""
    nc = _get_program()
    in_maps = _prepare_inputs(x, coeff, basis, bias)
    last_err = None
    for attempt in range(3):
        try:
            res = _bu.run_bass_kernel_spmd(
                nc,
                in_maps,
                list(range(N_CORES)),
                trace=trace,
                trace_cores=trace_cores,
            )
            return _assemble(res.results, x.shape[0]), res
        except Exception as e:  # transient NRT device-unrecoverable after
            last_err = e        # abrupt neighbor-process exits; nudge + retry
            if attempt == 2 or "UNAVAILABLE" not in str(e):
                raise
            import time

            import jax
            import jax.numpy as jnp

            time.sleep(15)
            try:
                a = jnp.ones((8, 8))
                (a @ a).block_until_ready()
            except Exception:
                time.sleep(15)
    raise last_err


def kernel(x, coeff, basis, bias):
    out, _ = run(x, coeff, basis, bias, trace=False)
    return out


# revision 3
# speedup vs baseline: 1.0654x; 1.0084x over previous
"""DISCO downsample conv (3x3, stride 2, pad 1) on 8 Trainium2 NeuronCores.

v4: K=128-packed matmuls — 8 matmuls per output row-pair instead of 9.

Baseline scheme: 9 tap-matmuls of K=96 (in-channels) per output row-pair;
TensorE streams 9 x 512 px = 4608 cycles/pair -> ~123us floor, PE only
96/128 partitions busy.

v4 packs one tap -- (2,2) -- into SBUF partitions 96-127 as host-prepared
shifted 32-channel copies ("helper"), so 3 of the matmuls contract
(96 ch x host tap) + (32 ch x a (2,2)-chunk) simultaneously: 3 x K=128 +
5 x K=96 = 8 matmuls/pair = 4096 cycles -> ~109us TensorE floor.

Why only one distributed tap: every matmul on the shared x tile reads at
byte offsets fixed by (kh row, kw column-base); with the pitch-514 phase
layout [evenPh 257 | oddPh 257], bases are {0, 257, 1} and kw=0/kw=2
ranges overlap, leaving exactly 4 non-colliding helper slots (2 even-row
+ 2 odd-row) -> 3 chunks of one tap. (A 7-matmul variant with 6 slots
needs a pitch-771 layout whose extra traffic / strided DMA destinations
measured slower: per-core DMA saturates at ~265 GB/s aggregate and
non-contiguous destinations collapse to tiny descriptors at <100 GB/s.)

Traffic: main x 12.7MB + helper 6.3MB + out 6.3MB per core, all
contiguous-destination transfers -> ~96us, under the 109us floor.

Numerics identical to baseline (same fp8 e3m4 x, fp16 w products, fp32
PSUM accumulation -- just regrouped), rel err ~1.26e-2.

Sharding: 8 shards = (batch 0..3) x (H half 0..1), 1-row halo included,
no inter-core communication. Startup warm-up matmuls, fine-grained
block-0 chunks, ACT-ring output flushes, and quartered last-block tail
kept from baseline.
"""

import os
import sys
import types

import numpy as np


# ----------------------------------------------------------------------------
# Environment bootstrap (self-contained: no reads from /root/problem).
# ----------------------------------------------------------------------------
def _ensure_paths():
    for p in (
        "/root/.axon_site",
        "/root/.axon_site/_ro/trn_rl_repo",
        "/root/.axon_site/_ro/pypackages",
        "/opt/trn_rl_repo",
    ):
        if os.path.isdir(p) and p not in sys.path:
            sys.path.append(p)


_ensure_paths()

import ml_dtypes  # noqa: E402


def _install_ntff_hook():
    """Register the NTFF profile hook (used when tracing; harmless otherwise)."""
    try:
        import antenv
    except ImportError:
        return
    if "antenv.axon_hooks" not in sys.modules:
        hooks_mod = types.ModuleType("antenv.axon_hooks")
        _hook = [None]
        hooks_mod.set_axon_ntff_profile_hook = lambda h: _hook.__setitem__(0, h)
        hooks_mod.get_axon_ntff_profile_hook = lambda: _hook[0]
        sys.modules["antenv.axon_hooks"] = hooks_mod
        antenv.axon_hooks = hooks_mod
    from antenv.axon_hooks import (
        get_axon_ntff_profile_hook,
        set_axon_ntff_profile_hook,
    )

    if get_axon_ntff_profile_hook() is None:
        try:
            from trn_agent_boot.trn_boot import _ntff_profile_via_ctypes

            so = "/opt/axon/libaxon_pjrt.so"
            if os.path.exists(so):
                set_axon_ntff_profile_hook(_ntff_profile_via_ctypes(so))
        except Exception:
            pass


_install_ntff_hook()

import concourse.bass as bass  # noqa: E402
import concourse.tile as tile  # noqa: E402
from concourse import bacc, mybir  # noqa: E402
import concourse.bass_utils as _bu  # noqa: E402

# Artifact upload needs a bucket that isn't reachable here; keep traces local.
_bu.upload_artifacts = lambda tmpdir: f"local:{tmpdir}"

XDT = mybir.dt.float8e3     # moving operand: fp8 e3m4 (4-bit mantissa)
WDT = mybir.dt.float16      # stationary weights stay fp16
F16 = mybir.dt.float16
F32 = mybir.dt.float32
NP_XDT = ml_dtypes.float8_e3m4
XSCALE = 4.0                # x *= 4 fits N(0,1) in e3m4 normals; w /= 4
XCLIP = 15.5                # e3m4 max finite

C = 96          # channels (in == out)
K = 3           # kernel size
N_CORES = 8
H = W = 512     # input spatial
HO = WO = 256   # output spatial
HP = H + 2      # padded rows
PITCH = 514     # cols: [evenPh 257 | oddPh 257]
SH_ROWS = 257   # padded rows per shard (256 + 1 halo)
CORE_HO = 128   # output rows per core
BH = 32         # output rows per block
NBLK = CORE_HO // BH
IN_ROWS = 2 * BH + 1  # input rows per block (65)
N_WARM = 55     # HAM pre-warm dummy matmuls (bridge to first-data ~17.5us)

# 8-matmul roster per row-pair: (kh, col_base, K_partitions, spare).
# col_base: kw=0 -> 0 (evenPh), kw=1 -> 257 (oddPh), kw=2 -> 1 (evenPh).
# K=128 entries carry a 32-ch chunk of the distributed tap (2,2) on
# partitions 96-127; spare = (tap, ch0) weight rows for those.
MM_ROSTER = (
    (0, 0, 96, None),        # (0,0)
    (0, 257, 128, (8, 32)),  # (0,1) + tap(2,2) ch 32-63
    (0, 1, 128, (8, 0)),     # (0,2) + tap(2,2) ch 0-31
    (1, 0, 96, None),        # (1,0)
    (1, 257, 128, (8, 64)),  # (1,1) + tap(2,2) ch 64-95
    (1, 1, 96, None),        # (1,2)
    (2, 0, 96, None),        # (2,0)
    (2, 257, 96, None),      # (2,1)
)
HOST_TAPS = (0, 1, 2, 3, 4, 5, 6, 7)  # tap index kh*3+kw per roster entry

_PROGRAM_CACHE = {}


def _build_program():
    """One SPMD Bass program, shared by all 8 cores."""
    nc = bacc.Bacc()
    x_d = nc.dram_tensor("x", [C, SH_ROWS, PITCH], XDT, kind="ExternalInput")
    h_d = nc.dram_tensor("hx", [32, SH_ROWS, PITCH], XDT, kind="ExternalInput")
    w_d = nc.dram_tensor("wt", [128, 8, C], WDT, kind="ExternalInput")
    b_d = nc.dram_tensor("bias", [C, 1], F32, kind="ExternalInput")
    y_d = nc.dram_tensor("out", [C, CORE_HO, WO], F16, kind="ExternalOutput")

    with tile.TileContext(nc) as tc:
        with (
            tc.tile_pool(name="const", bufs=1) as cpool,
            tc.tile_pool(name="xin", bufs=2) as xpool,
            tc.tile_pool(name="oout", bufs=2) as opool,
            tc.tile_pool(name="psum", bufs=8, space=bass.MemorySpace.PSUM) as ppool,
        ):
            # HAM pre-warm source: a zeroed fp16 tile, matmul'd into junk PSUM
            # while the first x chunk is still in DMA-queue-arm limbo.
            warm = cpool.tile([C, 352], F16)
            nc.vector.memset(warm[:], 0.0)
            # constants ride SWDGE: it emits right after the engine preamble
            # (~6us) and lands the small wt before the first x chunk arrives
            wt = cpool.tile([128, 8, C], WDT)
            nc.gpsimd.dma_start(wt[:], w_d[:])
            bias = cpool.tile([C, 1], F32)
            nc.gpsimd.dma_start(bias[:], b_d[:])

            # warm-up matmuls use full-size PSUM tiles so they don't perturb
            # the pool's bank packing for the real accumulators
            for i in range(N_WARM):
                wps = ppool.tile([C, 2 * WO], F32, name="wps", tag="ps")
                nc.tensor.matmul(
                    wps[:, 0:256], warm[:, 0:C], warm[:, C : C + 256],
                    start=True, stop=True,
                )

            for blk in range(NBLK):
                xt = xpool.tile([128, IN_ROWS, PITCH], XDT, name="xt", tag="xt")
                r0 = 2 * BH * blk
                # main channels (partitions 0-95) on the sync HWDGE ring
                chunks = (5, 13, 15, 16, 16) if blk == 0 else (33, 32)
                rr = 0
                for nrows in chunks:
                    nc.sync.dma_start(
                        xt[0:C, rr : rr + nrows, :],
                        x_d[:, r0 + rr : r0 + rr + nrows, :],
                    )
                    rr += nrows
                assert rr == IN_ROWS
                # helper partitions 96-127 on the gpsimd SWDGE ring
                hchunks = (5, 28, 32) if blk == 0 else (33, 32)
                rr = 0
                for nrows in hchunks:
                    nc.gpsimd.dma_start(
                        xt[C:128, rr : rr + nrows, :],
                        h_d[:, r0 + rr : r0 + rr + nrows, :],
                    )
                    rr += nrows
                assert rr == IN_ROWS
                out_sb = opool.tile([C, BH, WO], F16)
                last = blk == NBLK - 1
                # last block: final two rows become single-row N=256 tiles
                # below, halving the end-of-kernel ACT+flush chain
                for t in range(BH // 2 - 1 if last else BH // 2):
                    ps = ppool.tile([C, 2 * WO], F32, name="ps", tag="ps")
                    for k, (kh, cb, kp, _sp) in enumerate(MM_ROSTER):
                        rhs = xt[0:kp, 4 * t + kh : 4 * t + kh + 3 : 2, cb : cb + WO]
                        nc.tensor.matmul(
                            ps[:],
                            wt[0:kp, k, :],
                            rhs,
                            start=(k == 0),
                            stop=(k == len(MM_ROSTER) - 1),
                        )
                    nc.scalar.activation(
                        out_sb[:, 2 * t : 2 * t + 2, :],
                        ps[:].rearrange("p (a b) -> p a b", a=2),
                        mybir.ActivationFunctionType.Identity,
                        bias=bias[:],
                    )
                    # flush finished rows on the ACT HWDGE ring as soon as they
                    # complete; the last block flushes in quarters to shrink
                    # the kernel tail
                    flush_at = (
                        (7, 11, 13, 14) if last else (BH // 4 - 1, BH // 2 - 1)
                    )
                    if t in flush_at:
                        fi = flush_at.index(t)
                        prev = 0 if fi == 0 else (flush_at[fi - 1] + 1)
                        lo, hi = 2 * prev, 2 * t + 2
                        nc.scalar.dma_start(
                            y_d[:, BH * blk + lo : BH * blk + hi, :],
                            out_sb[:, lo:hi, :],
                        )
                if last:
                    for r in (BH - 2, BH - 1):
                        # full-bank PSUM tile (uniform 2KB allocations keep
                        # every accumulator bank-aligned)
                        psrf = ppool.tile([C, 2 * WO], F32, name="psrf", tag="ps")
                        psr = psrf[:, 0:WO]
                        for k, (kh, cb, kp, _sp) in enumerate(MM_ROSTER):
                            nc.tensor.matmul(
                                psr[:],
                                wt[0:kp, k, :],
                                xt[0:kp, 2 * r + kh, cb : cb + WO],
                                start=(k == 0),
                                stop=(k == len(MM_ROSTER) - 1),
                            )
                        nc.scalar.activation(
                            out_sb[:, r, :],
                            psr[:],
                            mybir.ActivationFunctionType.Identity,
                            bias=bias[:],
                        )
                        nc.scalar.dma_start(
                            y_d[:, BH * blk + r, :], out_sb[:, r, :]
                        )

    nc.compile()
    return nc


def _get_program():
    if "nc" not in _PROGRAM_CACHE:
        _PROGRAM_CACHE["nc"] = _build_program()
    return _PROGRAM_CACHE["nc"]


def _prepare_inputs(x, coeff, basis, bias):
    """Host prep: effective weights, phase-split e3m4 x + helper, shards."""
    x = np.asarray(x)
    coeff = np.asarray(coeff)
    basis = np.asarray(basis)
    bias = np.asarray(bias)
    B = coeff.shape[2]
    NB = x.shape[0]
    # wq[i, tap, o] = sum_b coeff[o,i,b] * basis[b, tap], pre-scaled 1/XSCALE
    w_eff = (
        coeff.astype(np.float32).reshape(C * C, B)
        @ basis.astype(np.float32).reshape(B, K * K)
    ).reshape(C, C, K * K)
    wq = np.ascontiguousarray(
        w_eff.transpose(1, 2, 0) * np.float32(1.0 / XSCALE)
    ).astype(np.float16)
    wt128 = np.zeros((128, 8, C), np.float16)
    for k, (kh, cb, kp, spare) in enumerate(MM_ROSTER):
        wt128[0:C, k, :] = wq[:, HOST_TAPS[k], :]
        if spare is not None:
            stap, ch0 = spare
            wt128[C:128, k, :] = wq[ch0 : ch0 + 32, stap, :]

    xs = np.clip(x.astype(np.float32) * np.float32(XSCALE), -XCLIP, XCLIP)
    xq = xs.astype(NP_XDT)
    # phases over padded cols: A[j] = padded col 2j, Bp[j] = padded col 2j+1
    A = np.zeros((NB, C, HP, 257), dtype=NP_XDT)
    Bp = np.zeros((NB, C, HP, 257), dtype=NP_XDT)
    A[:, :, 1 : H + 1, 1:257] = xq[:, :, :, 1::2]   # padded col 2j = orig 2j-1
    Bp[:, :, 1 : H + 1, 0:256] = xq[:, :, :, 0::2]  # padded col 2j+1 = orig 2j

    xph = np.zeros((NB, C, HP, PITCH), dtype=NP_XDT)
    xph[..., 0:257] = A
    xph[..., 257:514] = Bp

    # helper tensor: 3 slots carrying tap (2,2) chunks (see module doc)
    hlp = np.zeros((NB, 32, HP, PITCH), dtype=NP_XDT)
    ev = np.arange(0, HP, 2)
    ev2 = ev[ev + 2 < HP]
    od = np.arange(1, HP, 2)
    od1 = od[od + 1 < HP]
    hlp[:, :, ev2, 1:257] = A[:, 0:32, ev2 + 2, 1:257]     # (0,2) <- tap8 ch0
    hlp[:, :, ev2, 257:513] = A[:, 32:64, ev2 + 2, 1:257]  # (0,1) <- tap8 ch32
    hlp[:, :, od1, 257:513] = A[:, 64:96, od1 + 1, 1:257]  # (1,1) <- tap8 ch64

    bias2 = np.ascontiguousarray(bias.astype(np.float32).reshape(C, 1))

    in_maps = []
    for s in range(N_CORES):
        b_idx, h_idx = divmod(s, 2)
        shard = np.ascontiguousarray(
            xph[b_idx, :, 256 * h_idx : 256 * h_idx + SH_ROWS, :]
        )
        hshard = np.ascontiguousarray(
            hlp[b_idx, :, 256 * h_idx : 256 * h_idx + SH_ROWS, :]
        )
        in_maps.append(
            {"x": shard, "hx": hshard, "wt": wt128, "bias": bias2}
        )
    return in_maps


def _assemble(results, n_batch):
    out = np.empty((n_batch, C, 2 * CORE_HO, WO), dtype=np.float32)
    for s in range(N_CORES):
        b_idx, h_idx = divmod(s, 2)
        out[b_idx, :, CORE_HO * h_idx : CORE_HO * (h_idx + 1), :] = results[s][
            "out"
        ].astype(np.float32)
    return out


def run(x, coeff, basis, bias, trace=False, trace_cores=None):
    """Run the kernel; returns (full_output, BassKernelResults)."""
    nc = _get_program()
    in_maps = _prepare_inputs(x, coeff, basis, bias)
    last_err = None
    for attempt in range(3):
        try:
            res = _bu.run_bass_kernel_spmd(
                nc,
                in_maps,
                list(range(N_CORES)),
                trace=trace,
                trace_cores=trace_cores,
            )
            return _assemble(res.results, x.shape[0]), res
        except Exception as e:  # transient NRT device-unrecoverable after
            last_err = e        # abrupt neighbor-process exits; nudge + retry
            if attempt == 2 or "UNAVAILABLE" not in str(e):
                raise
            import time

            import jax
            import jax.numpy as jnp

            time.sleep(15)
            try:
                a = jnp.ones((8, 8))
                (a @ a).block_until_ready()
            except Exception:
                time.sleep(15)
    raise last_err


def kernel(x, coeff, basis, bias):
    out, _ = run(x, coeff, basis, bias, trace=False)
    return out
